# revision 5
# baseline (speedup 1.0000x reference)
"""Distributed spherical self-attention (DistributedAttentionS2) on 8 TRN2
NeuronCores.

Sharding: head-parallel (tensor parallel). 8 heads, 8 cores, one head per
core, no collectives.

The device kernel is PURE attention: the QKV projections, quadrature-weight
folding, and the output projection + softmax normalization run on the host
(rank-32 GEMMs — cheap on CPU; on-device they stole PE cycles, DVE copies,
and DMA bandwidth from the N^2 part).

Per-core device kernel (N = 4140 queries, NPAD = 4224 keys, dk = 32):
  - Inputs: Qrep [128, N] / Krep [128, NPAD] bf16 (the head's 32 channels
    replicated at partition bases 0/32/64/96 for 4-way PE row tiling),
    Vt [128, NKC, 33] bf16 (V^T pre-scaled by quadrature weights qw, with
    qw itself as column 32 so softmax denominators ride along).
  - Scores S^T [keys, queries] via bf16 matmuls (contraction 32), in
    2-key-chunk PSUM groups, triple-buffered so PE never waits on exp.
  - exp SPLIT across two engines, alternating per group:
      even groups -> ScalarE activation Exp (exact), bf16 out.
      odd groups  -> DVE Schraudolph: i16 = trunc(A*s + B) written through
        an int16-bitcast view of the bf16 et tile; the bf16 bit pattern IS
        2^((i - 127*128 + c)/128) ~= exp(SCALE*s) with ~2% sawtooth error.
        c = -7 zeroes the mean bias against the exact groups; measured
        end-to-end rel-l2 is ~6e-3 (gate 2e-2).
    This halves the ScalarE stream, the original bottleneck.
  - attnV: Vt per key-chunk against et, 2-way col-tiled (two query chunks
    at PSUM partition bases 0/64 of one bank), drained a few matmuls at a
    time between score groups. Chunk 0 (solo) and the last chunk's second
    strip run IN-chunk with a one-group lag so PE has attnV work even at
    the ends. Epilogue: PSUM->SBUF copy (DVE; DMA cannot read PSUM), then
    DMA A [33, 460] chunks to DRAM.
  - Host combine: out = p_w @ vstack_h(U_h / r_h) + p_w@v_b + p_b.
"""

import math

import numpy as np

HEADS = 8
C = 256
DK = 32
HLAT, WLON = 46, 90
N = HLAT * WLON  # 4140
NKC = 33  # key chunks of 128
NPAD = NKC * 128  # 4224
QCH = 460
NQC = 9  # 9 * 460 == 4140
SCALE = 1.0 / math.sqrt(DK)
EXP_A = SCALE * 128.0 * math.log2(math.e)
EXP_B = 127.0 * 128.0 - 7.0
# score groups: 16 of 2 key-chunks + 1 of 1  (33 kc total)
GROUPS = [(2 * g, min(2, NKC - 2 * g)) for g in range((NKC + 1) // 2)]

_cache = {}


def _build_nc():
    from contextlib import ExitStack

    import concourse.mybir as mybir
    import concourse.tile as tile
    from concourse import bacc

    f32 = mybir.dt.float32
    bf16 = mybir.dt.bfloat16
    i16 = mybir.dt.int16

    nc = bacc.Bacc("TRN2", target_bir_lowering=False, debug=False)

    qd = nc.dram_tensor("q", [128, N], bf16, kind="ExternalInput")
    kd = nc.dram_tensor("k", [128, NPAD], bf16, kind="ExternalInput")
    vd = nc.dram_tensor("v", [128, NKC, 33], bf16, kind="ExternalInput")
    ad = nc.dram_tensor("a", [33, N], f32, kind="ExternalOutput")

    with tile.TileContext(nc) as tc, ExitStack() as ctx:
        sing = ctx.enter_context(tc.tile_pool(name="sing", bufs=1))
        ets = ctx.enter_context(tc.tile_pool(name="ets", bufs=4))
        ous = ctx.enter_context(tc.tile_pool(name="ous", bufs=3))
        ps_s = ctx.enter_context(tc.tile_pool(name="ps_s", bufs=3, space="PSUM"))
        ps_o = ctx.enter_context(tc.tile_pool(name="ps_o", bufs=2, space="PSUM"))

        sb_q = sing.tile([128, N], bf16)
        sb_k = sing.tile([128, NPAD], bf16)
        sb_vt = sing.tile([128, NKC, 33], bf16)

        # Parallel input DMA across the three DMA-capable queues
        # (sync/gpsimd/scalar); the pieces the first score groups and
        # chunk-0 attnV need (K cols 0:768, vt, Q cols 0:460) go first on
        # separate queues so nothing serializes behind them.
        nc.sync.dma_start(out=sb_k[:, 0:768], in_=kd[:, 0:768])
        nc.gpsimd.dma_start(out=sb_vt[:], in_=vd[:])
        nc.gpsimd.dma_start(out=sb_q[:, 0:QCH], in_=qd[:, 0:QCH])
        nc.scalar.dma_start(out=sb_k[:, 768:2496], in_=kd[:, 768:2496])
        nc.scalar.dma_start(out=sb_k[:, 2496:NPAD], in_=kd[:, 2496:NPAD])
        nc.gpsimd.dma_start(out=sb_q[:, QCH : 5 * QCH], in_=qd[:, QCH : 5 * QCH])
        nc.sync.dma_start(out=sb_q[:, 5 * QCH : N], in_=qd[:, 5 * QCH : N])

        et_tiles = []
        avq = []  # pending emission closures (attnV MMs + epilogues)

        def drain(n):
            for _ in range(min(n, len(avq))):
                avq.pop(0)()

        def scores_and_exp(qc, tail_cb=None):
            et = ets.tile([128, NKC, QCH], bf16, tag="et")
            et_tiles.append(et)
            qsl = slice(qc * QCH, (qc + 1) * QCH)
            for g, (k0, nk) in enumerate(GROUPS):
                pg = ps_s.tile([128, 2, 512], f32, tag="s")
                for t in range(nk):
                    kc = k0 + t
                    base = 32 * (kc % 4)
                    nc.tensor.matmul(
                        pg[:, t, 0:QCH],
                        sb_k[base : base + 32, kc * 128 : (kc + 1) * 128],
                        sb_q[base : base + 32, qsl],
                        tile_position=(base, 0),
                    )
                if g % 2 == 1:
                    nc.vector.tensor_scalar(
                        out=et[:, k0 : k0 + nk, :].bitcast(i16),
                        in0=pg[:, 0:nk, 0:QCH],
                        scalar1=EXP_A,
                        scalar2=EXP_B,
                        op0=mybir.AluOpType.mult,
                        op1=mybir.AluOpType.add,
                    )
                else:
                    nc.scalar.activation(
                        out=et[:, k0 : k0 + nk, :],
                        in_=pg[:, 0:nk, 0:QCH],
                        func=mybir.ActivationFunctionType.Exp,
                        scale=SCALE,
                        bias=0.0,
                    )
                drain(3 if tail_cb is None else 5)
                if tail_cb is not None:
                    tail_cb(g)

        def av_pair_mm(jlo, box, kc, first=None, last=None):
            # attnV for qchunks (jlo, jlo+1): col-tiled strips at PSUM
            # partition bases 0 / 64 accumulating in one bank.
            first = 0 if first is None else first
            last = NKC - 1 if last is None else last
            if kc == first:
                box["po"] = ps_o.tile([128, 512], f32, tag="o", name="po_pair")
            po = box["po"]
            for s in range(2):
                base = 64 * s
                nc.tensor.matmul(
                    po[base : base + 33, 0:QCH],
                    sb_vt[:, kc, :],
                    et_tiles[jlo + s][:, kc, :],
                    start=(kc == first),
                    stop=(kc == last),
                    skip_group_check=True,
                )

        def av_pair_epi(jlo, box):
            po = box["po"]
            ou = ous.tile([128, QCH], f32, tag="ou")
            for s in range(2):
                base = 64 * s
                qc = jlo + s
                nc.vector.tensor_copy(
                    out=ou[base : base + 33, :],
                    in_=po[base : base + 33, 0:QCH],
                )
                eng = nc.sync if s == 0 else nc.gpsimd
                eng.dma_start(
                    out=ad[0:33, qc * QCH : (qc + 1) * QCH],
                    in_=ou[base : base + 33, :],
                )

        def enqueue_pair(jlo):
            box = {}
            for kc in range(NKC):
                avq.append(lambda kc=kc: av_pair_mm(jlo, box, kc))
            avq.append(lambda: av_pair_epi(jlo, box))

        H = QCH // 2  # 230

        # ---- chunk 0 (solo): attnV runs in-chunk with a one-group lag ----
        box0 = {}

        def av_solo_mm(kc):
            if kc == 0:
                box0["po"] = ps_o.tile([128, 512], f32, tag="o", name="po_solo")
            po = box0["po"]
            for s in range(2):
                base = 64 * s
                nc.tensor.matmul(
                    po[base : base + 33, 0:H],
                    sb_vt[:, kc, :],
                    et_tiles[0][:, kc, s * H : (s + 1) * H],
                    start=(kc == 0),
                    stop=(kc == NKC - 1),
                    skip_group_check=True,
                )

        def av_solo_epi():
            po = box0["po"]
            ou = ous.tile([128, QCH], f32, tag="ou")
            for s in range(2):
                base = 64 * s
                nc.vector.tensor_copy(
                    out=ou[base : base + 33, 0:H], in_=po[base : base + 33, 0:H]
                )
                eng = nc.sync if s == 0 else nc.gpsimd
                eng.dma_start(
                    out=ad[0:33, s * H : (s + 1) * H],
                    in_=ou[base : base + 33, 0:H],
                )

        def solo_cb(g):
            if g >= 1:
                k0, nk = GROUPS[g - 1]
                for kc in range(k0, k0 + nk):
                    av_solo_mm(kc)

        scores_and_exp(0, solo_cb)
        k0, nk = GROUPS[-1]
        for kc in range(k0, k0 + nk):
            avq.append(lambda kc=kc: av_solo_mm(kc))
        avq.append(av_solo_epi)

        scores_and_exp(1)
        for qc in range(2, NQC):
            if qc in (3, 5, 7):  # pairs (1,2), (3,4), (5,6)
                enqueue_pair(qc - 2)
            if qc == NQC - 1:
                # Last pair (7, 8) is split per strip: qc7's strip (ET7
                # complete) drains via the queue and retires early; qc8's
                # strip follows exp8 with a one-group lag, kc order
                # [2..32, 0..1] so the final matmuls have no exp dependency.
                box8 = {}

                def strip_mm(s, kc, first, last):
                    base = 64 * s
                    if "po" not in box8:
                        box8["po"] = ps_o.tile(
                            [128, 512], f32, tag="o", name="po_last"
                        )
                    po = box8["po"]
                    nc.tensor.matmul(
                        po[base : base + 33, 0:QCH],
                        sb_vt[:, kc, :],
                        et_tiles[NQC - 2 + s][:, kc, :],
                        start=(kc == first),
                        stop=(kc == last),
                        skip_group_check=True,
                    )

                def epi_strip(s):
                    qcs = NQC - 2 + s
                    base = 64 * s
                    po = box8["po"]
                    ou = ous.tile([128, QCH], f32, tag="ou", name="ou_l")
                    nc.vector.tensor_copy(
                        out=ou[base : base + 33, :],
                        in_=po[base : base + 33, 0:QCH],
                    )
                    eng = nc.sync if s == 0 else nc.gpsimd
                    eng.dma_start(
                        out=ad[0:33, qcs * QCH : (qcs + 1) * QCH],
                        in_=ou[base : base + 33, :],
                    )

                for kc in range(NKC):
                    avq.append(lambda kc=kc: strip_mm(0, kc, 0, NKC - 1))
                avq.append(lambda: epi_strip(0))

                def tail_cb(g):
                    # skip GROUPS[0] here: kc 2 carries the accumulation
                    # start flag, so it must be the first strip-1 matmul;
                    # kcs 32, 0, 1 are emitted after the loop (kc 1 stops).
                    if g >= 2:
                        k0, nk = GROUPS[g - 1]
                        for kc in range(k0, k0 + nk):
                            strip_mm(1, kc, 2, 1)

                scores_and_exp(qc, tail_cb)
            else:
                scores_and_exp(qc)
        drain(len(avq))
        k0, nk = GROUPS[-1]
        for kc in list(range(k0, k0 + nk)) + [0, 1]:
            strip_mm(1, kc, 2, 1)
        epi_strip(1)

    nc.compile()
    return nc


def _host_inputs(query, q_w, q_b, k_w, k_b, v_w, log_qw):
    import ml_dtypes

    bf = ml_dtypes.bfloat16
    xb = np.asarray(query, dtype=np.float32).reshape(C, N).astype(bf).astype(
        np.float32
    )

    lq = np.asarray(log_qw, dtype=np.float32).reshape(N).astype(np.float64)
    lq = lq - lq.max()  # global shift cancels in U/r
    qw = np.exp(lq)
    qw_pad = np.zeros(NPAD, np.float64)
    qw_pad[:N] = qw

    in_maps = []
    for h in range(HEADS):
        hs = slice(DK * h, DK * (h + 1))
        wq = np.asarray(q_w, np.float32)[hs].astype(bf).astype(np.float32)
        wk = np.asarray(k_w, np.float32)[hs].astype(bf).astype(np.float32)
        wv = np.asarray(v_w, np.float32)[hs].astype(bf).astype(np.float32)

        q = wq @ xb + np.asarray(q_b, np.float32)[hs][:, None]
        k = wk @ xb + np.asarray(k_b, np.float32)[hs][:, None]
        v = wv @ xb  # v_b folded on the host combine side

        qrep = np.ascontiguousarray(np.tile(q.astype(bf), (4, 1)))
        kp = np.zeros((DK, NPAD), np.float32)
        kp[:, :N] = k
        krep = np.ascontiguousarray(np.tile(kp.astype(bf), (4, 1)))

        vt = np.zeros((NPAD, 33), np.float32)
        vt[:N, 0:32] = (v * qw[None, :]).T
        vt[:, 32] = qw_pad
        vtl = np.ascontiguousarray(
            vt.astype(bf).reshape(NKC, 128, 33).transpose(1, 0, 2)
        )

        in_maps.append({"q": qrep, "k": krep, "v": vtl})
    return in_maps


def kernel(query, q_w, q_b, k_w, k_b, v_w, v_b, p_w, p_b, log_qw, _res=None):
    from concourse.bass_utils import run_bass_kernel_spmd

    if "nc" not in _cache:
        _cache["nc"] = _build_nc()
    nc = _cache["nc"]

    in_maps = _host_inputs(query, q_w, q_b, k_w, k_b, v_w, log_qw)
    res = run_bass_kernel_spmd(nc, in_maps, core_ids=list(range(8)))
    if _res is not None:
        _res.append(res)

    P = np.empty((C, N), np.float64)
    for h in range(HEADS):
        a = res.results[h]["a"].astype(np.float64)
        P[DK * h : DK * (h + 1)] = a[0:32] / a[32][None, :]

    out = np.asarray(p_w, np.float64) @ P
    out += (np.asarray(p_w, np.float64) @ np.asarray(v_b, np.float64))[:, None]
    out += np.asarray(p_b, np.float64)[:, None]
    return out.astype(np.float32).reshape(1, C, HLAT, WLON)


# revision 6
# speedup vs baseline: 1.0226x; 1.0226x over previous
"""Distributed spherical self-attention (DistributedAttentionS2) on 8 TRN2
NeuronCores.

Sharding: head-parallel (tensor parallel). 8 heads, 8 cores, one head per
core, no collectives.

The device kernel is PURE attention: the QKV projections, quadrature-weight
folding, and the output projection + softmax normalization run on the host
(rank-32 GEMMs — cheap on CPU; on-device they stole PE cycles, DVE copies,
and DMA bandwidth from the N^2 part).

Per-core device kernel (N = 4140 queries, NPAD = 4224 keys, dk = 32):
  - Inputs: Qrep [128, N] / Krep [128, NPAD] bf16 (the head's 32 channels
    replicated at partition bases 0/32/64/96 for 4-way PE row tiling),
    Vt [128, NKC, 33] bf16 (V^T pre-scaled by quadrature weights qw, with
    qw itself as column 32 so softmax denominators ride along).
  - Scores S^T [keys, queries] via bf16 matmuls (contraction 32), 3-chunk
    PSUM groups, double buffered.
  - exp of every group SPLIT BY COLUMNS across two engines running
    concurrently (halves the PSUM-free latency so PE never waits):
      cols 0:252   -> ScalarE activation Exp (exact), bf16 out.
      cols 252:460 -> DVE Schraudolph: i16 = trunc(A*s + B) written through
        an int16-bitcast view of the bf16 et tile; the bf16 bit pattern IS
        2^((i - 127*128 + c)/128) ~= exp(SCALE*s) with ~2% sawtooth error.
        c = -7 zeroes the mean bias against the exact columns; measured
        end-to-end rel-l2 is ~6e-3 (gate 2e-2).
    This also halves the ScalarE stream, the original bottleneck.
  - attnV: Vt per key-chunk against et, 2-way col-tiled (two query chunks
    at PSUM partition bases 0/64 of one bank), drained a few matmuls at a
    time between score groups. Chunk 0 (solo) and the last chunk's second
    strip run IN-chunk with a one-group lag so PE has attnV work at the
    ends too. Epilogue: PSUM->SBUF copy (DVE; DMA cannot read PSUM), then
    DMA A [33, 460] chunks to DRAM.
  - Host combine: out = p_w @ vstack_h(U_h / r_h) + p_w@v_b + p_b.
"""

import math

import numpy as np

HEADS = 8
C = 256
DK = 32
HLAT, WLON = 46, 90
N = HLAT * WLON  # 4140
NKC = 33  # key chunks of 128
NPAD = NKC * 128  # 4224
QCH = 460
NQC = 9  # 9 * 460 == 4140
CSP = 252  # exp column split: ScalarE [0:CSP], DVE [CSP:QCH]
SCALE = 1.0 / math.sqrt(DK)
EXP_A = SCALE * 128.0 * math.log2(math.e)
EXP_B = 127.0 * 128.0 - 7.0

_cache = {}


def _build_nc():
    from contextlib import ExitStack

    import concourse.mybir as mybir
    import concourse.tile as tile
    from concourse import bacc

    f32 = mybir.dt.float32
    bf16 = mybir.dt.bfloat16
    i16 = mybir.dt.int16

    nc = bacc.Bacc("TRN2", target_bir_lowering=False, debug=False)

    qd = nc.dram_tensor("q", [128, N], bf16, kind="ExternalInput")
    kd = nc.dram_tensor("k", [128, NPAD], bf16, kind="ExternalInput")
    vd = nc.dram_tensor("v", [128, NKC, 33], bf16, kind="ExternalInput")
    ad = nc.dram_tensor("a", [33, N], f32, kind="ExternalOutput")

    with tile.TileContext(nc) as tc, ExitStack() as ctx:
        sing = ctx.enter_context(tc.tile_pool(name="sing", bufs=1))
        ets = ctx.enter_context(tc.tile_pool(name="ets", bufs=4))
        ous = ctx.enter_context(tc.tile_pool(name="ous", bufs=3))
        ps_s = ctx.enter_context(tc.tile_pool(name="ps_s", bufs=2, space="PSUM"))
        ps_o = ctx.enter_context(tc.tile_pool(name="ps_o", bufs=2, space="PSUM"))

        sb_q = sing.tile([128, N], bf16)
        sb_k = sing.tile([128, NPAD], bf16)
        sb_vt = sing.tile([128, NKC, 33], bf16)
        warm = sing.tile([128, 8], f32)

        # Warm the ScalarE Exp activation table (1.3us load) during the
        # input-DMA dead time instead of on the first real exp.
        nc.gpsimd.memset(warm[:], 0.0)
        nc.scalar.activation(
            out=warm[:],
            in_=warm[:],
            func=mybir.ActivationFunctionType.Exp,
            scale=1.0,
            bias=0.0,
        )

        # Input DMA: K on sync, everything else on gpsimd (ScalarE/DVE are
        # the exp engines and DVE can't issue DMAs anyway). Pieces ordered
        # by first use: K cols 0:768 + vt + Q cols 0:460 land first.
        nc.sync.dma_start(out=sb_k[:, 0:768], in_=kd[:, 0:768])
        nc.gpsimd.dma_start(out=sb_vt[:], in_=vd[:])
        nc.gpsimd.dma_start(out=sb_q[:, 0:QCH], in_=qd[:, 0:QCH])
        nc.sync.dma_start(out=sb_k[:, 768:2496], in_=kd[:, 768:2496])
        nc.sync.dma_start(out=sb_k[:, 2496:NPAD], in_=kd[:, 2496:NPAD])
        nc.gpsimd.dma_start(out=sb_q[:, QCH : 5 * QCH], in_=qd[:, QCH : 5 * QCH])
        nc.gpsimd.dma_start(out=sb_q[:, 5 * QCH : N], in_=qd[:, 5 * QCH : N])

        et_tiles = []
        avq = []  # pending emission closures (attnV MMs + epilogues)

        def drain(n):
            for _ in range(min(n, len(avq))):
                avq.pop(0)()

        def scores_and_exp(qc, tail_cb=None):
            et = ets.tile([128, NKC, QCH], bf16, tag="et")
            et_tiles.append(et)
            qsl = slice(qc * QCH, (qc + 1) * QCH)
            for g in range(11):
                pg = ps_s.tile([128, 3, 512], f32, tag="s")
                for t in range(3):
                    kc = 3 * g + t
                    base = 32 * (kc % 4)
                    nc.tensor.matmul(
                        pg[:, t, 0:QCH],
                        sb_k[base : base + 32, kc * 128 : (kc + 1) * 128],
                        sb_q[base : base + 32, qsl],
                        tile_position=(base, 0),
                    )
                nc.scalar.activation(
                    out=et[:, 3 * g : 3 * g + 3, 0:CSP],
                    in_=pg[:, :, 0:CSP],
                    func=mybir.ActivationFunctionType.Exp,
                    scale=SCALE,
                    bias=0.0,
                )
                nc.vector.tensor_scalar(
                    out=et[:, 3 * g : 3 * g + 3, CSP:QCH].bitcast(i16),
                    in0=pg[:, :, CSP:QCH],
                    scalar1=EXP_A,
                    scalar2=EXP_B,
                    op0=mybir.AluOpType.mult,
                    op1=mybir.AluOpType.add,
                )
                drain(5 if tail_cb is None else 7)
                if tail_cb is not None:
                    tail_cb(g)

        def av_pair_mm(jlo, box, kc, first=None, last=None):
            # attnV for qchunks (jlo, jlo+1): col-tiled strips at PSUM
            # partition bases 0 / 64 accumulating in one bank.
            first = 0 if first is None else first
            last = NKC - 1 if last is None else last
            if kc == first:
                box["po"] = ps_o.tile([128, 512], f32, tag="o", name="po_pair")
            po = box["po"]
            for s in range(2):
                base = 64 * s
                nc.tensor.matmul(
                    po[base : base + 33, 0:QCH],
                    sb_vt[:, kc, :],
                    et_tiles[jlo + s][:, kc, :],
                    start=(kc == first),
                    stop=(kc == last),
                    skip_group_check=True,
                )

        def av_pair_epi(jlo, box):
            po = box["po"]
            ou = ous.tile([128, QCH], f32, tag="ou")
            for s in range(2):
                base = 64 * s
                qc = jlo + s
                nc.vector.tensor_copy(
                    out=ou[base : base + 33, :],
                    in_=po[base : base + 33, 0:QCH],
                )
                eng = nc.sync if s == 0 else nc.gpsimd
                eng.dma_start(
                    out=ad[0:33, qc * QCH : (qc + 1) * QCH],
                    in_=ou[base : base + 33, :],
                )

        def enqueue_pair(jlo):
            box = {}
            for kc in range(NKC):
                avq.append(lambda kc=kc: av_pair_mm(jlo, box, kc))
            avq.append(lambda: av_pair_epi(jlo, box))

        H = QCH // 2  # 230

        # ---- chunk 0 (solo): attnV runs in-chunk with a one-group lag ----
        box0 = {}

        def av_solo_mm(kc):
            if kc == 0:
                box0["po"] = ps_o.tile([128, 512], f32, tag="o", name="po_solo")
            po = box0["po"]
            for s in range(2):
                base = 64 * s
                nc.tensor.matmul(
                    po[base : base + 33, 0:H],
                    sb_vt[:, kc, :],
                    et_tiles[0][:, kc, s * H : (s + 1) * H],
                    start=(kc == 0),
                    stop=(kc == NKC - 1),
                    skip_group_check=True,
                )

        def av_solo_epi():
            po = box0["po"]
            ou = ous.tile([128, QCH], f32, tag="ou")
            for s in range(2):
                base = 64 * s
                nc.vector.tensor_copy(
                    out=ou[base : base + 33, 0:H], in_=po[base : base + 33, 0:H]
                )
                eng = nc.sync if s == 0 else nc.gpsimd
                eng.dma_start(
                    out=ad[0:33, s * H : (s + 1) * H],
                    in_=ou[base : base + 33, 0:H],
                )

        def solo_cb(g):
            if g >= 1:
                for kc in range(3 * (g - 1), 3 * g):
                    av_solo_mm(kc)

        scores_and_exp(0, solo_cb)
        for kc in range(30, NKC):
            avq.append(lambda kc=kc: av_solo_mm(kc))
        avq.append(av_solo_epi)

        scores_and_exp(1)
        for qc in range(2, NQC):
            if qc in (3, 5, 7):  # pairs (1,2), (3,4), (5,6)
                enqueue_pair(qc - 2)
            if qc == NQC - 1:
                # Last pair (7, 8) is split per strip: qc7's strip (ET7
                # complete) drains via the queue and retires early; qc8's
                # strip follows exp8 with a one-group lag, kc order
                # [3..32, 0..2] so the final matmuls have no exp dependency.
                box8 = {}

                def strip_mm(s, kc, first, last):
                    base = 64 * s
                    if "po" not in box8:
                        box8["po"] = ps_o.tile(
                            [128, 512], f32, tag="o", name="po_last"
                        )
                    po = box8["po"]
                    nc.tensor.matmul(
                        po[base : base + 33, 0:QCH],
                        sb_vt[:, kc, :],
                        et_tiles[NQC - 2 + s][:, kc, :],
                        start=(kc == first),
                        stop=(kc == last),
                        skip_group_check=True,
                    )

                def epi_strip(s):
                    qcs = NQC - 2 + s
                    base = 64 * s
                    po = box8["po"]
                    ou = ous.tile([128, QCH], f32, tag="ou", name="ou_l")
                    nc.vector.tensor_copy(
                        out=ou[base : base + 33, :],
                        in_=po[base : base + 33, 0:QCH],
                    )
                    eng = nc.sync if s == 0 else nc.gpsimd
                    eng.dma_start(
                        out=ad[0:33, qcs * QCH : (qcs + 1) * QCH],
                        in_=ou[base : base + 33, :],
                    )

                for kc in range(NKC):
                    avq.append(lambda kc=kc: strip_mm(0, kc, 0, NKC - 1))
                avq.append(lambda: epi_strip(0))

                def tail_cb(g):
                    if g >= 2:
                        for kc in range(3 * (g - 1), 3 * g):
                            strip_mm(1, kc, 3, 2)

                scores_and_exp(qc, tail_cb)
            else:
                scores_and_exp(qc)
        drain(len(avq))
        for kc in list(range(30, NKC)) + [0, 1, 2]:
            strip_mm(1, kc, 3, 2)
        epi_strip(1)

    nc.compile()
    return nc


def _host_inputs(query, q_w, q_b, k_w, k_b, v_w, log_qw):
    import ml_dtypes

    bf = ml_dtypes.bfloat16
    xb = np.asarray(query, dtype=np.float32).reshape(C, N).astype(bf).astype(
        np.float32
    )

    lq = np.asarray(log_qw, dtype=np.float32).reshape(N).astype(np.float64)
    lq = lq - lq.max()  # global shift cancels in U/r
    qw = np.exp(lq)
    qw_pad = np.zeros(NPAD, np.float64)
    qw_pad[:N] = qw

    in_maps = []
    for h in range(HEADS):
        hs = slice(DK * h, DK * (h + 1))
        wq = np.asarray(q_w, np.float32)[hs].astype(bf).astype(np.float32)
        wk = np.asarray(k_w, np.float32)[hs].astype(bf).astype(np.float32)
        wv = np.asarray(v_w, np.float32)[hs].astype(bf).astype(np.float32)

        q = wq @ xb + np.asarray(q_b, np.float32)[hs][:, None]
        k = wk @ xb + np.asarray(k_b, np.float32)[hs][:, None]
        v = wv @ xb  # v_b folded on the host combine side

        qrep = np.ascontiguousarray(np.tile(q.astype(bf), (4, 1)))
        kp = np.zeros((DK, NPAD), np.float32)
        kp[:, :N] = k
        krep = np.ascontiguousarray(np.tile(kp.astype(bf), (4, 1)))

        vt = np.zeros((NPAD, 33), np.float32)
        vt[:N, 0:32] = (v * qw[None, :]).T
        vt[:, 32] = qw_pad
        vtl = np.ascontiguousarray(
            vt.astype(bf).reshape(NKC, 128, 33).transpose(1, 0, 2)
        )

        in_maps.append({"q": qrep, "k": krep, "v": vtl})
    return in_maps


def kernel(query, q_w, q_b, k_w, k_b, v_w, v_b, p_w, p_b, log_qw, _res=None):
    from concourse.bass_utils import run_bass_kernel_spmd

    if "nc" not in _cache:
        _cache["nc"] = _build_nc()
    nc = _cache["nc"]

    in_maps = _host_inputs(query, q_w, q_b, k_w, k_b, v_w, log_qw)
    res = run_bass_kernel_spmd(nc, in_maps, core_ids=list(range(8)))
    if _res is not None:
        _res.append(res)

    P = np.empty((C, N), np.float64)
    for h in range(HEADS):
        a = res.results[h]["a"].astype(np.float64)
        P[DK * h : DK * (h + 1)] = a[0:32] / a[32][None, :]

    out = np.asarray(p_w, np.float64) @ P
    out += (np.asarray(p_w, np.float64) @ np.asarray(v_b, np.float64))[:, None]
    out += np.asarray(p_b, np.float64)[:, None]
    return out.astype(np.float32).reshape(1, C, HLAT, WLON)
